# revision 1
# baseline (speedup 1.0000x reference)
"""Trainium2 Bass kernel for nn_MultiHeadAttention (B=4, T=2048, D=1024, H=16).

Sharding: 8 cores = 4 batches x 2 query-halves. Each core runs the full
attention for its 1024 queries against all 2048 keys (all 16 heads), so no
cross-core communication is needed; the host only concatenates the 8 output
slices. Odd cores receive a row-permuted x (their query half first) so the
same program runs on every core; attention is permutation-invariant over keys.

On-core layout: everything is computed feature-major ("transposed") so the
softmax feeds the PE directly:
  x^T (d on partitions)  ->  Q^T, K^T (feature-major), V (token-major)
  S^T[k, q] = K_h^T.T @ Q_h^T   (PE, contraction over head_dim=64,
                                 head pairs packed in partition halves)
  P^T = exp(S^T / 8)            (ACT, straight out of PSUM)
  O^T[d, q] += V_tile.T @ P^T   (PE, two heads packed via column groups)
  denom = ones.T @ sum_k P^T    (DVE accumulate + PE partition-sum)
  out[t, :] = (O^T / denom).T @ W_out + b_out
Matmuls run in float32r (single-pass PE mode, ~1.5e-4 rel err).
"""

import sys

sys.path.insert(0, "/opt/trn_rl_repo")

import numpy as np

B, T, D = 4, 2048, 1024
H, HD = 16, 64
NCORES = 8
TQ = T // 2  # queries per core
NP = 128
KT = T // NP  # 16 key tiles
DC = D // NP  # 8 d_model chunks
PAIRS = H // 2  # 8 head pairs; pair p owns features [128p, 128p+128)

_CACHE = {}


def _build():
    import concourse.bacc as bacc
    import concourse.tile as tile
    from concourse import masks, mybir

    F32 = mybir.dt.float32
    F32R = mybir.dt.float32r
    BF16 = mybir.dt.bfloat16
    AF = mybir.ActivationFunctionType

    nc = bacc.Bacc("TRN2", target_bir_lowering=False, debug=False,
                   num_devices=NCORES)
    x_io = nc.dram_tensor("x", [T, D], F32, kind="ExternalInput").ap()
    wqkv_io = nc.dram_tensor("wqkv", [D, 3 * D], F32, kind="ExternalInput").ap()
    bqkv_io = nc.dram_tensor("bqkv", [3 * D], F32, kind="ExternalInput").ap()
    wout_io = nc.dram_tensor("wout", [D, D], F32, kind="ExternalInput").ap()
    bout_io = nc.dram_tensor("bout", [D], F32, kind="ExternalInput").ap()
    out_io = nc.dram_tensor("out", [TQ, D], F32, kind="ExternalOutput").ap()

    qspill = nc.dram_tensor("qspill", [D, TQ], F32R).ap()  # Q^T feature-major
    kspill = nc.dram_tensor("kspill", [D, T], F32R).ap()   # K^T feature-major
    vspill = nc.dram_tensor("vspill", [T, D], BF16).ap()   # V token-major (bf16)

    bq_col = bqkv_io.rearrange("(n o) -> n o", o=1)  # [3072, 1]
    bq_row = bqkv_io.rearrange("(o n) -> o n", o=1)  # [1, 3072]
    bout_row = bout_io.rearrange("(o n) -> o n", o=1)  # [1, 1024]

    with tile.TileContext(nc) as tc:
        with (
            tc.tile_pool(name="const", bufs=1) as cpool,
            tc.tile_pool(name="otres", bufs=1) as ot_pool,
        ):
            ident = cpool.tile([NP, NP], F32, name="ident")
            masks.make_identity(nc, ident[:])
            ones_row = cpool.tile([1, NP], F32, name="ones_row")
            nc.vector.memset(ones_row[:], 1.0)
            ones_col = cpool.tile([NP, 1], BF16, name="ones_col")
            nc.vector.memset(ones_col[:], 1.0)
            # ind16[P][h, c] = 1.0 iff head h of pair P owns column c
            ones64 = cpool.tile([1, 64], F32, name="ones64")
            nc.vector.memset(ones64[:], 1.0)
            ind16 = []
            for p in range(PAIRS):
                t = cpool.tile([H, NP], F32, name=f"ind16_{p}")
                nc.vector.memset(t[:], 0.0)
                nc.sync.dma_start(t[2 * p:2 * p + 1, 0:64], ones64[:])
                nc.sync.dma_start(t[2 * p + 1:2 * p + 2, 64:NP], ones64[:])
                ind16.append(t)

            # b_v and b_out broadcast to [128, D] via K=1 ones matmul
            bv_row_sb = cpool.tile([1, D], F32, name="bv_row_sb")
            nc.sync.dma_start(bv_row_sb[:], bq_row[:, 2 * D:3 * D])
            bo_row_sb = cpool.tile([1, D], F32, name="bo_row_sb")
            nc.sync.dma_start(bo_row_sb[:], bout_row[:])
            bv_bc = cpool.tile([NP, D], F32, name="bv_bc")
            bo_bc = cpool.tile([NP, D], F32, name="bo_bc")
            with tc.tile_pool(name="bc_ps", bufs=2, space="PSUM") as bc_ps_pool:
                for dst, src in ((bv_bc, bv_row_sb), (bo_bc, bo_row_sb)):
                    for c in range(2):
                        ps = bc_ps_pool.tile([NP, 512], F32, name="bcps", tag="bcps")
                        nc.tensor.matmul(ps[:], ones_row[:], src[:, c * 512:(c + 1) * 512])
                        nc.vector.tensor_copy(dst[:, c * 512:(c + 1) * 512], ps[:])

            # ---- Stage A: x -> x^T (f32r), via PE transpose ----
            # xT_all[p, dc*T + t] = x[t, dc*128 + p]
            with (
                tc.tile_pool(name="xT", bufs=1) as xT_pool,
                nc.named_scope("xT"),
            ):
                xT = xT_pool.tile([NP, DC * T], F32R, name="xT")
                xT_v = xT.rearrange("p (dc t) -> p dc t", dc=DC)
                with (
                    tc.tile_pool(name="xload", bufs=3) as xl_pool,
                    tc.tile_pool(name="tr_ps", bufs=3, space="PSUM") as tr_pool,
                ):
                    for j in range(KT):  # 16 t-tiles
                        xt = xl_pool.tile([NP, D], F32, name="xt", tag="xt")
                        nc.sync.dma_start(xt[:], x_io[j * NP:(j + 1) * NP, :])
                        ps = tr_pool.tile([NP, D], F32, name="trps", tag="trps")
                        for dc in range(DC):
                            nc.tensor.transpose(
                                ps[:, dc * NP:(dc + 1) * NP],
                                xt[:, dc * NP:(dc + 1) * NP], ident[:])
                        nc.vector.tensor_copy(
                            xT_v[:, :, j * NP:(j + 1) * NP],
                            ps.rearrange("p (dc t) -> p dc t", dc=DC))

                # ---- Stage B: QKV projections ----
                xT_f = xT_v  # [128, dc, t]
                with (
                    nc.named_scope("qkv"),
                    tc.tile_pool(name="wload", bufs=3) as wl_pool,
                    tc.tile_pool(name="wr", bufs=10) as wr_pool,
                    tc.tile_pool(name="bias", bufs=2) as bias_pool,
                    tc.tile_pool(name="qkv_ps", bufs=6, space="PSUM") as qkv_ps_pool,
                    tc.tile_pool(name="qkv_sb", bufs=2) as qkv_sb_pool,
                ):
                    # Q^T and K^T (feature-major): lhsT = W tile, rhs = x^T
                    for is_k in (False, True):
                        ncols = T if is_k else TQ
                        f0 = D if is_k else 0
                        spill = kspill if is_k else qspill
                        nm = "k" if is_k else "q"
                        for p in range(PAIRS):
                            wts = []
                            for dc in range(DC):
                                wtmp = wl_pool.tile([NP, NP], F32, name=f"w{nm}l{p}_{dc}", tag="wl")
                                nc.sync.dma_start(
                                    wtmp[:],
                                    wqkv_io[dc * NP:(dc + 1) * NP,
                                            f0 + p * NP:f0 + (p + 1) * NP])
                                wr = wr_pool.tile([NP, NP], F32R, name=f"w{nm}r{p}_{dc}", tag="wr")
                                nc.vector.tensor_copy(wr[:], wtmp[:])
                                wts.append(wr)
                            bias = bias_pool.tile([NP, 1], F32, name=f"b{nm}{p}", tag="bias")
                            nc.sync.dma_start(
                                bias[:], bq_col[f0 + p * NP:f0 + (p + 1) * NP, :])
                            sb = qkv_sb_pool.tile([NP, ncols], F32R, name=f"{nm}sb{p}", tag=f"sb{nm}")
                            nch = ncols // 512
                            pss = [qkv_ps_pool.tile([NP, 512], F32, name=f"{nm}ps{p}_{c}", tag="qkvps")
                                   for c in range(nch)]
                            for dc in range(DC):  # one weight load, nch matmuls
                                for c in range(nch):
                                    nc.tensor.matmul(
                                        pss[c][:], wts[dc][:],
                                        xT_f[:, dc, c * 512:(c + 1) * 512],
                                        start=(dc == 0), stop=(dc == DC - 1))
                            for c in range(nch):
                                nc.vector.tensor_scalar_add(
                                    sb[:, c * 512:(c + 1) * 512], pss[c][:], bias[:])
                            nc.sync.dma_start(
                                spill[p * NP:(p + 1) * NP, :], sb[:])

                    # V (token-major): lhsT = x^T tile (one load serves both
                    # 512-wide f chunks), rhs = W_v columns
                    wvs = []
                    for dc in range(DC):
                        wtmp = wl_pool.tile([NP, D], F32, name=f"wvl{dc}", tag="wvl")
                        nc.sync.dma_start(
                            wtmp[:], wqkv_io[dc * NP:(dc + 1) * NP, 2 * D:3 * D])
                        wr = wr_pool.tile([NP, D], F32R, name=f"wvr{dc}", tag="wvr")
                        nc.vector.tensor_copy(wr[:], wtmp[:])
                        wvs.append(wr)
                    for ti in range(KT):
                        pss = [qkv_ps_pool.tile([NP, 512], F32, name=f"vps{ti}_{c}", tag="qkvps")
                               for c in range(2)]
                        for dc in range(DC):
                            for c in range(2):
                                nc.tensor.matmul(
                                    pss[c][:], xT_f[:, dc, ti * NP:(ti + 1) * NP],
                                    wvs[dc][:, c * 512:(c + 1) * 512],
                                    start=(dc == 0), stop=(dc == DC - 1))
                        for c in range(2):
                            sb = qkv_sb_pool.tile([NP, 512], BF16, name=f"vsb{ti}_{c}", tag="sbv")
                            nc.vector.tensor_add(
                                sb[:], pss[c][:], bv_bc[:, c * 512:(c + 1) * 512])
                            nc.sync.dma_start(
                                vspill[ti * NP:(ti + 1) * NP,
                                       c * 512:(c + 1) * 512], sb[:])

            # ---- Stage C: attention per head pair ----
            oT = [ot_pool.tile([NP, TQ], F32R, name=f"oT{p}") for p in range(PAIRS)]
            vsp_v = vspill.rearrange("(i tp) f -> tp i f", tp=NP)
            with (
                nc.named_scope("attn"),
                tc.tile_pool(name="qt", bufs=2) as qt_pool,
                tc.tile_pool(name="kt", bufs=2) as kt_pool,
                tc.tile_pool(name="vt", bufs=2) as vt_pool,
                tc.tile_pool(name="pt", bufs=4) as pt_pool,
                tc.tile_pool(name="acc", bufs=2) as acc_pool,
                tc.tile_pool(name="rcp", bufs=1) as rcp_pool,
                tc.tile_pool(name="s_ps", bufs=3, space="PSUM") as s_pool,
                tc.tile_pool(name="o_ps", bufs=1, space="PSUM") as o_pool,
            ):
                recip_in = rcp_pool.tile([H, TQ], F32, name="recip_in")
                recip_out = rcp_pool.tile([H, TQ], F32, name="recip_out")
                for p in range(PAIRS):
                    qt = qt_pool.tile([NP, TQ], F32R, name=f"qt{p}", tag="qt")
                    nc.sync.dma_start(qt[:], qspill[p * NP:(p + 1) * NP, :])
                    kt = kt_pool.tile([NP, T], F32R, name=f"kt{p}", tag="kt")
                    nc.sync.dma_start(kt[:], kspill[p * NP:(p + 1) * NP, :])
                    vt = vt_pool.tile([NP, KT * NP], BF16, name=f"vt{p}", tag="vt")
                    nc.sync.dma_start(
                        vt.rearrange("tp (i c) -> tp i c", i=KT),
                        vsp_v[:, :, p * NP:(p + 1) * NP])

                    ops = o_pool.tile([NP, TQ], F32, name=f"ops{p}", tag="ops")
                    accA = acc_pool.tile([NP, TQ], BF16, name=f"accA{p}", tag="accA")
                    accB = acc_pool.tile([NP, TQ], BF16, name=f"accB{p}", tag="accB")
                    for i in range(KT):
                        sA = s_pool.tile([NP, TQ], F32, name=f"sA{p}_{i}", tag="s")
                        sB = s_pool.tile([NP, TQ], F32, name=f"sB{p}_{i}", tag="s")
                        for c in range(2):
                            nc.tensor.matmul(
                                sA[:, c * 512:(c + 1) * 512],
                                kt[0:HD, i * NP:(i + 1) * NP],
                                qt[0:HD, c * 512:(c + 1) * 512])
                        for c in range(2):
                            nc.tensor.matmul(
                                sB[:, c * 512:(c + 1) * 512],
                                kt[HD:NP, i * NP:(i + 1) * NP],
                                qt[HD:NP, c * 512:(c + 1) * 512])
                        pA = pt_pool.tile([NP, TQ], BF16, name=f"pA{p}_{i}", tag="pt")
                        pB = pt_pool.tile([NP, TQ], BF16, name=f"pB{p}_{i}", tag="pt")
                        nc.scalar.activation(pA[:], sA[:], AF.Exp, scale=0.125)
                        nc.scalar.activation(pB[:], sB[:], AF.Exp, scale=0.125)
                        if i == 0:
                            nc.vector.tensor_copy(accA[:], pA[:])
                            nc.vector.tensor_copy(accB[:], pB[:])
                        else:
                            nc.vector.tensor_add(accA[:], accA[:], pA[:])
                            nc.vector.tensor_add(accB[:], accB[:], pB[:])
                        # col-packed heads share PSUM banks; the sim's
                        # bank-granular group check false-positives here
                        for c in range(2):
                            nc.tensor.matmul(
                                ops[0:HD, c * 512:(c + 1) * 512],
                                vt[:, i * NP:i * NP + HD],
                                pA[:, c * 512:(c + 1) * 512],
                                start=(i == 0), stop=(i == KT - 1),
                                skip_group_check=True)
                        for c in range(2):
                            nc.tensor.matmul(
                                ops[HD:NP, c * 512:(c + 1) * 512],
                                vt[:, i * NP + HD:(i + 1) * NP],
                                pB[:, c * 512:(c + 1) * 512],
                                start=(i == 0), stop=(i == KT - 1),
                                skip_group_check=True)

                    nc.vector.tensor_copy(oT[p][:], ops[:])
                    # denominators: partition-sum of acc via ones matmul,
                    # DMA the [1, TQ] rows straight out of PSUM; the
                    # reciprocal + normalize run once after all pairs.
                    for h, acc in ((0, accA), (1, accB)):
                        dn = o_pool.tile([NP, TQ], F32, name=f"dn{p}_{h}", tag="ops")
                        for c in range(2):
                            nc.tensor.matmul(
                                dn[0:1, c * 512:(c + 1) * 512], ones_col[:],
                                acc[:, c * 512:(c + 1) * 512])
                        dnr = rcp_pool.tile([1, TQ], F32, name=f"dnr{p}_{h}",
                                            tag="dnr", bufs=4)
                        nc.vector.tensor_copy(dnr[:], dn[0:1, :])
                        nc.sync.dma_start(recip_in[2 * p + h:2 * p + h + 1, :],
                                          dnr[:])

                # deferred normalization: one batched reciprocal, then
                # per-pair broadcast matmul + multiply
                nc.vector.reciprocal(recip_out[:], recip_in[:])
                for p in range(PAIRS):
                    rbc = s_pool.tile([NP, TQ], F32, name=f"rbc{p}", tag="s")
                    for c in range(2):
                        nc.tensor.matmul(
                            rbc[:, c * 512:(c + 1) * 512], ind16[p][:],
                            recip_out[:, c * 512:(c + 1) * 512])
                    nc.vector.tensor_mul(oT[p][:], oT[p].bitcast(F32)[:], rbc[:])

            # ---- Stage D: out projection ----
            with (
                nc.named_scope("outproj"),
                tc.tile_pool(name="wo", bufs=1) as wo_pool,
                tc.tile_pool(name="wol", bufs=2) as wol_pool,
                tc.tile_pool(name="f_ps", bufs=4, space="PSUM") as f_ps_pool,
                tc.tile_pool(name="f_sb", bufs=3) as f_sb_pool,
            ):
                wos = []
                for p in range(PAIRS):
                    wtmp = wol_pool.tile([NP, D], F32, name=f"wol{p}", tag="wol")
                    nc.sync.dma_start(wtmp[:], wout_io[p * NP:(p + 1) * NP, :])
                    wo = wo_pool.tile([NP, D], F32R, name=f"wo{p}")
                    nc.vector.tensor_copy(wo[:], wtmp[:])
                    wos.append(wo)
                for tj in range(TQ // NP):
                    fsb = f_sb_pool.tile([NP, D], F32, name=f"fsb{tj}", tag="fsb")
                    pss = [f_ps_pool.tile([NP, 512], F32, name=f"fps{tj}_{c}", tag="fps")
                           for c in range(2)]
                    for p in range(PAIRS):
                        for c in range(2):
                            nc.tensor.matmul(
                                pss[c][:], oT[p][:, tj * NP:(tj + 1) * NP],
                                wos[p][:, c * 512:(c + 1) * 512],
                                start=(p == 0), stop=(p == PAIRS - 1))
                    for c in range(2):
                        nc.vector.tensor_add(
                            fsb[:, c * 512:(c + 1) * 512], pss[c][:],
                            bo_bc[:, c * 512:(c + 1) * 512])
                    nc.sync.dma_start(out_io[tj * NP:(tj + 1) * NP, :], fsb[:])

    nc.compile()
    return nc


def get_nc():
    if "nc" not in _CACHE:
        _CACHE["nc"] = _build()
    return _CACHE["nc"]


def make_in_maps(x, W_qkv, b_qkv, W_out, b_out):
    x = np.ascontiguousarray(np.asarray(x, dtype=np.float32))
    W_qkv = np.ascontiguousarray(np.asarray(W_qkv, dtype=np.float32))
    b_qkv = np.ascontiguousarray(np.asarray(b_qkv, dtype=np.float32))
    W_out = np.ascontiguousarray(np.asarray(W_out, dtype=np.float32))
    b_out = np.ascontiguousarray(np.asarray(b_out, dtype=np.float32))
    in_maps = []
    for core in range(NCORES):
        b, half = divmod(core, 2)
        xb = x[b]
        if half == 1:  # put this core's query rows first; key order is free
            xb = np.concatenate([xb[TQ:], xb[:TQ]], axis=0)
        in_maps.append({
            "x": np.ascontiguousarray(xb),
            "wqkv": W_qkv, "bqkv": b_qkv, "wout": W_out, "bout": b_out,
        })
    return in_maps


def run(in_maps, trace=False):
    from concourse.bass_utils import run_bass_kernel_spmd
    nc = get_nc()
    return run_bass_kernel_spmd(nc, in_maps, list(range(NCORES)), trace=trace)


def kernel(x, W_qkv, b_qkv, W_out, b_out):
    res = run(make_in_maps(x, W_qkv, b_qkv, W_out, b_out))
    out = np.empty((B, T, D), dtype=np.float32)
    for core in range(NCORES):
        b, half = divmod(core, 2)
        out[b, half * TQ:(half + 1) * TQ] = res.results[core]["out"]
    return out



# revision 21
# speedup vs baseline: 2.1413x; 2.1413x over previous
"""Trainium2 Bass kernel for nn_MultiHeadAttention (B=4, T=2048, D=1024, H=16).

Sharding: 8 cores = 4 batches x 2 query-halves. Each core runs the full
attention for its 1024 queries against all 2048 keys (all 16 heads); the host
concatenates the 8 output slices. Odd cores receive a row-permuted x (their
query half first) so one program serves every core.

v2 design (vs the spill-based v1): everything stays in SBUF.
  Phase A+B (fused per token tile): x -> x^T (f32r, PE transpose) and
    V = x@Wv+bv (token-major bf16, f32r matmuls, 512-col chunks).
  Phase C: per (head-pair, 512-query chunk) attention blocks.
    S^T[k,q] bf16 matmuls (two heads packed side by side in one [128,1024]
    PSUM tile), one exp per block-iter on ACT (the throughput floor),
    denominator = DVE bf16 accumulation + ones-matmul partition sum,
    O^T accumulated in a 1-bank PSUM tile, normalization folded into the
    PSUM drain. Q/K projections for the NEXT pair are interleaved into the
    PE stream so the tensor engine never idles (keeps the 2.4 GHz pstate).
  Phase D: out = (O^T/denom)^T @ W_out + b_out, bf16 matmuls.
Weights are DMA'd directly as float32r (bit-identical to f32) - no casts.
"""

import sys

sys.path.insert(0, "/opt/trn_rl_repo")

import numpy as np

B, T, D = 4, 2048, 1024
H, HD = 16, 64
NCORES = 8
TQ = T // 2  # queries per core
NP = 128
KT = T // NP  # 16 key tiles
DC = D // NP  # 8 d_model chunks
PAIRS = H // 2  # 8 head pairs; pair p owns features [128p, 128p+128)
QC = TQ // 512  # 2 query chunks per core

_CACHE = {}


def _build():
    from contextlib import ExitStack

    import concourse.bacc as bacc
    import concourse.tile as tile
    from concourse import masks, mybir

    F32 = mybir.dt.float32
    F32R = mybir.dt.float32r
    BF16 = mybir.dt.bfloat16
    AF = mybir.ActivationFunctionType

    nc = bacc.Bacc("TRN2", target_bir_lowering=False, debug=False,
                   num_devices=NCORES)
    x_io = nc.dram_tensor("x", [T, D], F32, kind="ExternalInput").ap()
    wqkv_io = nc.dram_tensor("wqkv", [D, 3 * D], F32R, kind="ExternalInput").ap()
    bqkv_io = nc.dram_tensor("bqkv", [3 * D], F32, kind="ExternalInput").ap()
    wout_io = nc.dram_tensor("wout", [D, D], F32, kind="ExternalInput").ap()
    bout_io = nc.dram_tensor("bout", [D], F32, kind="ExternalInput").ap()
    out_io = nc.dram_tensor("out", [TQ, D], F32, kind="ExternalOutput").ap()

    bq_col = bqkv_io.rearrange("(n o) -> n o", o=1)  # [3072, 1]
    bq_row = bqkv_io.rearrange("(o n) -> o n", o=1)  # [1, 3072]
    bout_row = bout_io.rearrange("(o n) -> o n", o=1)  # [1, 1024]
    # [r, dc, e]: row r of d-chunk dc, output feature e
    wq_view = wqkv_io.rearrange("(dc r) e -> r dc e", r=NP)
    wout_view = wout_io.rearrange("(p r) e -> r p e", r=NP)

    with ExitStack() as stack:
        tc = stack.enter_context(tile.TileContext(nc))
        cpool = stack.enter_context(tc.tile_pool(name="const", bufs=1))
        v_pool = stack.enter_context(tc.tile_pool(name="vfull", bufs=1))
        ot_pool = stack.enter_context(tc.tile_pool(name="oT", bufs=1))
        qt_pool = stack.enter_context(tc.tile_pool(name="qt", bufs=2))
        kt_pool = stack.enter_context(tc.tile_pool(name="kt", bufs=2))
        wqk_pool = stack.enter_context(tc.tile_pool(name="wqk", bufs=2))
        bias_pool = stack.enter_context(tc.tile_pool(name="bias", bufs=4))
        if True:
            # ---- constants ----
            ident = cpool.tile([NP, NP], F32, name="ident")
            masks.make_identity(nc, ident[:])
            ones_row = cpool.tile([1, NP], F32, name="ones_row")
            nc.vector.memset(ones_row[:], 1.0)
            ones_col = cpool.tile([NP, 1], BF16, name="ones_col")
            nc.vector.memset(ones_col[:], 1.0)
            ones64f = cpool.tile([1, 64], F32, name="ones64f")
            nc.vector.memset(ones64f[:], 1.0)
            # ind65[h, r] = 1.0 iff the half owning output row r has its
            # reciprocal on partition h (0 or 64); rows 1..63 are zero so the
            # zero-initialized filler rows of rec65 contribute nothing.
            ind65f = cpool.tile([65, NP], F32, name="ind65f")
            nc.vector.memset(ind65f[:], 0.0)
            nc.sync.dma_start(ind65f[0:1, 0:64], ones64f[:])
            nc.sync.dma_start(ind65f[64:65, 64:NP], ones64f[:])
            ind65 = cpool.tile([65, NP], F32R, name="ind65")
            nc.vector.tensor_copy(ind65[:], ind65f[:])
            # persistent reciprocal staging rows (0 and 64); filler rows stay 0
            rec65f = cpool.tile([65, 512], F32, name="rec65f")
            nc.vector.memset(rec65f[:], 0.0)
            rec65 = cpool.tile([65, 512], F32R, name="rec65")
            nc.vector.tensor_copy(rec65[:], rec65f[:])
            recBf = cpool.tile([1, 512], F32, name="recBf")

            # b_v and b_out broadcast to [128, D] via K=1 ones matmul
            bv_row_sb = cpool.tile([1, D], F32, name="bv_row_sb")
            nc.sync.dma_start(bv_row_sb[:], bq_row[:, 2 * D:3 * D])
            bo_row_sb = cpool.tile([1, D], F32, name="bo_row_sb")
            nc.sync.dma_start(bo_row_sb[:], bout_row[:])
            bv_bc = cpool.tile([NP, D], F32, name="bv_bc")
            bo_bc = cpool.tile([NP, D], F32, name="bo_bc")
            with tc.tile_pool(name="bc_ps", bufs=2, space="PSUM") as bc_ps_pool:
                for dst, src in ((bv_bc, bv_row_sb), (bo_bc, bo_row_sb)):
                    for c in range(2):
                        ps = bc_ps_pool.tile([NP, 512], F32, name="bcps", tag="bcps")
                        nc.tensor.matmul(ps[:], ones_row[:], src[:, c * 512:(c + 1) * 512])
                        nc.vector.tensor_copy(dst[:, c * 512:(c + 1) * 512], ps[:])

            # ---- long-lived SBUF tensors ----
            vfull = v_pool.tile([NP, KT * D], BF16, name="vfull")
            v_v = vfull.rearrange("p (i f) -> p i f", i=KT)  # [tok128, ktile, feat]
            oT = [ot_pool.tile([NP, TQ], BF16, name=f"oT{p}") for p in range(PAIRS)]

            with tc.tile_pool(name="xT", bufs=1) as xT_pool:
                xT = xT_pool.tile([NP, DC * T], F32R, name="xT")
                xT_v = xT.rearrange("p (dc t) -> p dc t", dc=DC)

                # ---- Phase A+B: transpose x, compute V (wv halves c-outer) ----
                with ExitStack() as hstack:
                    hstack.enter_context(nc.named_scope("head"))
                    wv_pool = hstack.enter_context(tc.tile_pool(name="wv", bufs=1))
                    xl_pool = hstack.enter_context(tc.tile_pool(name="xload", bufs=2))
                    tr_pool = hstack.enter_context(
                        tc.tile_pool(name="tr_ps", bufs=2, space="PSUM"))
                    v_ps_pool = hstack.enter_context(
                        tc.tile_pool(name="v_ps", bufs=4, space="PSUM"))
                    for c in range(2):
                        wv = wv_pool.tile([NP, DC * 512], F32R, name=f"wv{c}", tag="wv")
                        wv_v = wv.rearrange("p (dc f) -> p dc f", dc=DC)
                        nc.sync.dma_start(
                            wv_v[:], wq_view[:, :, 2 * D + c * 512:2 * D + (c + 1) * 512])
                        for ti in range(KT):
                            if c == 0:
                                xt = xl_pool.tile([NP, D], F32, name=f"xt{ti}", tag="xt")
                                nc.sync.dma_start(xt[:], x_io[ti * NP:(ti + 1) * NP, :])
                                ps = tr_pool.tile([NP, D], F32, name=f"trps{ti}", tag="trps")
                                for dc in range(DC):
                                    nc.tensor.transpose(
                                        ps[:, dc * NP:(dc + 1) * NP],
                                        xt[:, dc * NP:(dc + 1) * NP], ident[:])
                                nc.vector.tensor_copy(
                                    xT_v[:, :, ti * NP:(ti + 1) * NP],
                                    ps.rearrange("p (dc t) -> p dc t", dc=DC))
                            vps = v_ps_pool.tile([NP, 512], F32, name=f"vps{c}_{ti}", tag="vps")
                            for dc in range(DC):
                                nc.tensor.matmul(
                                    vps[:], xT_v[:, dc, ti * NP:(ti + 1) * NP],
                                    wv_v[:, dc, :],
                                    start=(dc == 0), stop=(dc == DC - 1))
                            nc.vector.tensor_add(
                                v_v[:, ti, c * 512:(c + 1) * 512], vps[:],
                                bv_bc[:, c * 512:(c + 1) * 512])

                # ---- Phase C: fused QK projection + attention ----
                with ExitStack() as astack:
                    astack.enter_context(nc.named_scope("attn"))
                    wobf_pool = astack.enter_context(tc.tile_pool(name="wout_bf", bufs=1))
                    wol_pool = astack.enter_context(tc.tile_pool(name="wol", bufs=2))
                    pab_pool = astack.enter_context(tc.tile_pool(name="pab", bufs=5))
                    acc_pool = astack.enter_context(tc.tile_pool(name="acc", bufs=2))
                    rbcsb_pool = astack.enter_context(tc.tile_pool(name="rbcsb", bufs=2))
                    s_pool = astack.enter_context(
                        tc.tile_pool(name="s_ps", bufs=2, space="PSUM"))
                    ops_pool = astack.enter_context(
                        tc.tile_pool(name="ops_ps", bufs=2, space="PSUM"))
                    dn_pool = astack.enter_context(
                        tc.tile_pool(name="dn_ps", bufs=1, space="PSUM"))
                    proj_pool = astack.enter_context(
                        tc.tile_pool(name="proj_ps", bufs=1, space="PSUM"))
                    wout_bf = wobf_pool.tile([NP, PAIRS * D], BF16, name="wout_bf")
                    wout_bfv = wout_bf.rearrange("r (p e) -> r p e", p=PAIRS)

                    qt_tiles, kt_tiles = {}, {}

                    def alloc_pair(p):
                        """DMA W/bias for pair p and allocate its q^T/k^T tiles."""
                        wt = wqk_pool.tile([NP, DC * 256], F32R, name=f"wqk{p}", tag="wqk")
                        wt_v = wt.rearrange("r (dc f) -> r dc f", dc=DC)
                        nc.sync.dma_start(wt_v[:, :, 0:128],
                                          wq_view[:, :, p * NP:(p + 1) * NP])
                        nc.sync.dma_start(wt_v[:, :, 128:256],
                                          wq_view[:, :, D + p * NP:D + (p + 1) * NP])
                        bq = bias_pool.tile([NP, 1], F32, name=f"bq{p}", tag="bq")
                        nc.sync.dma_start(bq[:], bq_col[p * NP:(p + 1) * NP, :])
                        bk = bias_pool.tile([NP, 1], F32, name=f"bk{p}", tag="bk")
                        nc.sync.dma_start(bk[:], bq_col[D + p * NP:D + (p + 1) * NP, :])
                        qt_tiles[p] = qt_pool.tile([NP, TQ], BF16, name=f"qt{p}", tag="qt")
                        kt_tiles[p] = kt_pool.tile([NP, T], BF16, name=f"kt{p}", tag="kt")
                        return wt_v, bq, bk

                    def proj_chunk(p, wt_v, bq, bk, chunk):
                        """One 512-token chunk of pair-p Q^T or K^T projection."""
                        is_k = chunk >= QC
                        j = chunk - QC if is_k else chunk
                        off, dst, bias = ((128, kt_tiles[p], bk) if is_k
                                          else (0, qt_tiles[p], bq))
                        ps = proj_pool.tile([NP, 512], F32, name=f"pj{p}_{chunk}", tag="pj")
                        for dc in range(DC):
                            nc.tensor.matmul(
                                ps[:], wt_v[:, dc, off:off + 128],
                                xT_v[:, dc, j * 512:(j + 1) * 512],
                                start=(dc == 0), stop=(dc == DC - 1))
                        nc.vector.tensor_scalar_add(
                            dst[:, j * 512:(j + 1) * 512], ps[:], bias[:])

                    def emit_s(p, qc, i):
                        """S^T for both heads of pair p: [128 keys, 512+512 q]."""
                        s = s_pool.tile([NP, 1024], F32, name=f"s{p}_{qc}_{i}", tag="s")
                        nc.tensor.matmul(
                            s[:, 0:512], kt_tiles[p][0:HD, i * NP:(i + 1) * NP],
                            qt_tiles[p][0:HD, qc * 512:(qc + 1) * 512])
                        nc.tensor.matmul(
                            s[:, 512:1024], kt_tiles[p][HD:NP, i * NP:(i + 1) * NP],
                            qt_tiles[p][HD:NP, qc * 512:(qc + 1) * 512])
                        return s

                    # pair 0 projections run up front
                    pair_state = {0: alloc_pair(0)}
                    for chunk in range(3 * QC):
                        proj_chunk(0, *pair_state[0], chunk)

                    def boundary_stages(p, qc, acc, ops):
                        """Denominator -> reciprocal -> broadcast -> normalized
                        drain for block (p, qc), split into stages that run
                        interleaved with the NEXT block so nothing stalls."""
                        def stage_a():
                            dnA = dn_pool.tile([NP, 512], F32,
                                               name=f"dnA{p}_{qc}", tag="dn")
                            nc.tensor.matmul(dnA[0:1, :], ones_col[:], acc[:, 0:512])
                            nc.vector.reciprocal_approx_fast(rec65f[0:1, :], dnA[0:1, :])

                        def stage_b():
                            dnB = dn_pool.tile([NP, 512], F32,
                                               name=f"dnB{p}_{qc}", tag="dn")
                            nc.tensor.matmul(dnB[0:1, :], ones_col[:], acc[:, 512:1024])
                            nc.vector.reciprocal_approx_fast(recBf[:], dnB[0:1, :])
                            nc.sync.dma_start(rec65f[64:65, :], recBf[:])
                            with nc.allow_low_precision(reason="f32r recip bcast"):
                                nc.vector.tensor_copy(rec65[:], rec65f[:])

                        def stage_c():
                            rbc = dn_pool.tile([NP, 512], F32,
                                               name=f"rbc{p}_{qc}", tag="dn")
                            nc.tensor.matmul(rbc[:], ind65[:], rec65[:])
                            rbc_sb = rbcsb_pool.tile([NP, 512], F32,
                                                     name=f"rbs{p}_{qc}", tag="rbs")
                            nc.vector.tensor_copy(rbc_sb[:], rbc[:])
                            nc.vector.tensor_mul(
                                oT[p][:, qc * 512:(qc + 1) * 512], ops[:], rbc_sb[:])

                        return {2: stage_a, 3: stage_b, 6: stage_c}

                    blocks = [(p, qc) for p in range(PAIRS) for qc in range(QC)]
                    s_early = None
                    pending = {}
                    for bi, (p, qc) in enumerate(blocks):
                        if qc == 0 and p + 1 < PAIRS:
                            pair_state[p + 1] = alloc_pair(p + 1)
                        ops = ops_pool.tile([NP, 512], F32, name=f"ops{p}_{qc}", tag="ops")
                        acc = acc_pool.tile([NP, 1024], BF16, name=f"acc{p}_{qc}", tag="acc")
                        for i in range(KT):
                            s = s_early if (i == 0 and s_early is not None) \
                                else emit_s(p, qc, i)
                            s_early = None
                            pab = pab_pool.tile([NP, 1024], BF16,
                                                name=f"pab{p}_{qc}_{i}", tag="pab")
                            nc.scalar.activation(pab[:], s[:], AF.Exp, scale=0.125)
                            if i == 0:
                                nc.vector.tensor_copy(acc[:], pab[:])
                            else:
                                nc.vector.tensor_add(acc[:], acc[:], pab[:])
                            # two heads share the ops PSUM bank at different
                            # partition offsets -> skip bank-granular group check
                            nc.tensor.matmul(
                                ops[0:HD, :], v_v[:, i, p * NP:p * NP + HD],
                                pab[:, 0:512],
                                start=(i == 0), stop=(i == KT - 1),
                                skip_group_check=True)
                            nc.tensor.matmul(
                                ops[HD:NP, :], v_v[:, i, p * NP + HD:(p + 1) * NP],
                                pab[:, 512:1024],
                                start=(i == 0), stop=(i == KT - 1),
                                skip_group_check=True)
                            # previous block's deferred denominator stages
                            if i in pending:
                                pending.pop(i)()
                            # keep the PE fed: next pair's projections + wout cast
                            if qc == 0 and p + 1 < PAIRS and i in (4, 5, 7, 8, 10, 12):
                                proj_chunk(p + 1, *pair_state[p + 1],
                                           (4, 5, 7, 8, 10, 12).index(i))
                            if qc == 1 and i == 9:
                                wol = wol_pool.tile([NP, D], F32, name=f"wol{p}", tag="wol")
                                nc.sync.dma_start(wol[:], wout_view[:, p, :])
                                nc.vector.tensor_copy(wout_bfv[:, p, :], wol[:])
                        # next block's first S goes ahead of the boundary ops so
                        # ACT never waits on the denominator chain
                        if bi + 1 < len(blocks):
                            np_, nqc = blocks[bi + 1]
                            s_early = emit_s(np_, nqc, 0)
                            pending = boundary_stages(p, qc, acc, ops)
                        else:
                            for fn in boundary_stages(p, qc, acc, ops).values():
                                fn()

            # ---- Phase D: out projection ----
            with ExitStack() as dstack:
                dstack.enter_context(nc.named_scope("outproj"))
                f_ps_pool = dstack.enter_context(
                    tc.tile_pool(name="f_ps", bufs=4, space="PSUM"))
                f_sb_pool = dstack.enter_context(tc.tile_pool(name="f_sb", bufs=3))
                for tj in range(TQ // NP):
                    fsb = f_sb_pool.tile([NP, D], F32, name=f"fsb{tj}", tag="fsb")
                    for c in range(2):
                        ps = f_ps_pool.tile([NP, 512], F32, name=f"fps{tj}_{c}", tag="fps")
                        for p in range(PAIRS):
                            nc.tensor.matmul(
                                ps[:], oT[p][:, tj * NP:(tj + 1) * NP],
                                wout_bfv[:, p, c * 512:(c + 1) * 512],
                                start=(p == 0), stop=(p == PAIRS - 1))
                        nc.vector.tensor_add(
                            fsb[:, c * 512:(c + 1) * 512], ps[:],
                            bo_bc[:, c * 512:(c + 1) * 512])
                    nc.sync.dma_start(out_io[tj * NP:(tj + 1) * NP, :], fsb[:])

    nc.compile()
    return nc


def get_nc():
    if "nc" not in _CACHE:
        _CACHE["nc"] = _build()
    return _CACHE["nc"]


def make_in_maps(x, W_qkv, b_qkv, W_out, b_out):
    x = np.ascontiguousarray(np.asarray(x, dtype=np.float32))
    W_qkv = np.ascontiguousarray(np.asarray(W_qkv, dtype=np.float32))
    b_qkv = np.ascontiguousarray(np.asarray(b_qkv, dtype=np.float32))
    W_out = np.ascontiguousarray(np.asarray(W_out, dtype=np.float32))
    b_out = np.ascontiguousarray(np.asarray(b_out, dtype=np.float32))
    in_maps = []
    for core in range(NCORES):
        b, half = divmod(core, 2)
        xb = x[b]
        if half == 1:  # put this core's query rows first; key order is free
            xb = np.concatenate([xb[TQ:], xb[:TQ]], axis=0)
        in_maps.append({
            "x": np.ascontiguousarray(xb),
            "wqkv": W_qkv, "bqkv": b_qkv, "wout": W_out, "bout": b_out,
        })
    return in_maps


def run(in_maps, trace=False):
    from concourse.bass_utils import run_bass_kernel_spmd
    nc = get_nc()
    return run_bass_kernel_spmd(nc, in_maps, list(range(NCORES)), trace=trace)


def kernel(x, W_qkv, b_qkv, W_out, b_out):
    res = run(make_in_maps(x, W_qkv, b_qkv, W_out, b_out))
    out = np.empty((B, T, D), dtype=np.float32)
    for core in range(NCORES):
        b, half = divmod(core, 2)
        out[b, half * TQ:(half + 1) * TQ] = res.results[core]["out"]
    return out


# revision 25
# speedup vs baseline: 2.2849x; 1.0671x over previous
"""Trainium2 Bass kernel for nn_MultiHeadAttention (B=4, T=2048, D=1024, H=16).

Sharding: 8 cores = 4 batches x 2 query-halves. Each core runs the full
attention for its 1024 queries against all 2048 keys (all 16 heads); the host
concatenates the 8 output slices. Odd cores receive a row-permuted x (their
query half first) so one program serves every core.

v2 design (vs the spill-based v1): everything stays in SBUF.
  Phase A+B (fused per token tile): x -> x^T (f32r, PE transpose) and
    V = x@Wv+bv (token-major bf16, f32r matmuls, 512-col chunks).
  Phase C: per (head-pair, 512-query chunk) attention blocks.
    S^T[k,q] bf16 matmuls (two heads packed side by side in one [128,1024]
    PSUM tile), one exp per block-iter on ACT (the throughput floor),
    denominator = DVE bf16 accumulation + ones-matmul partition sum,
    O^T accumulated in a 1-bank PSUM tile, normalization folded into the
    PSUM drain. Q/K projections for the NEXT pair are interleaved into the
    PE stream so the tensor engine never idles (keeps the 2.4 GHz pstate).
  Phase D: out = (O^T/denom)^T @ W_out + b_out, bf16 matmuls.
Weights are DMA'd directly as float32r (bit-identical to f32) - no casts.
"""

import sys

sys.path.insert(0, "/opt/trn_rl_repo")

import numpy as np

B, T, D = 4, 2048, 1024
H, HD = 16, 64
NCORES = 8
TQ = T // 2  # queries per core
NP = 128
KT = T // NP  # 16 key tiles
DC = D // NP  # 8 d_model chunks
PAIRS = H // 2  # 8 head pairs; pair p owns features [128p, 128p+128)
QC = TQ // 512  # 2 query chunks per core

_CACHE = {}


def _build():
    from contextlib import ExitStack

    import concourse.bacc as bacc
    import concourse.tile as tile
    from concourse import masks, mybir

    F32 = mybir.dt.float32
    F32R = mybir.dt.float32r
    BF16 = mybir.dt.bfloat16
    AF = mybir.ActivationFunctionType

    nc = bacc.Bacc("TRN2", target_bir_lowering=False, debug=False,
                   num_devices=NCORES)
    x_io = nc.dram_tensor("x", [T, D], F32, kind="ExternalInput").ap()
    wqkv_io = nc.dram_tensor("wqkv", [D, 3 * D], F32R, kind="ExternalInput").ap()
    bqkv_io = nc.dram_tensor("bqkv", [3 * D], F32, kind="ExternalInput").ap()
    wout_io = nc.dram_tensor("wout", [D, D], F32, kind="ExternalInput").ap()
    bout_io = nc.dram_tensor("bout", [D], F32, kind="ExternalInput").ap()
    out_io = nc.dram_tensor("out", [TQ, D], F32, kind="ExternalOutput").ap()

    bq_col = bqkv_io.rearrange("(n o) -> n o", o=1)  # [3072, 1]
    bq_row = bqkv_io.rearrange("(o n) -> o n", o=1)  # [1, 3072]
    bout_row = bout_io.rearrange("(o n) -> o n", o=1)  # [1, 1024]
    # [r, dc, e]: row r of d-chunk dc, output feature e
    wq_view = wqkv_io.rearrange("(dc r) e -> r dc e", r=NP)
    wout_view = wout_io.rearrange("(p r) e -> r p e", r=NP)

    with ExitStack() as stack:
        tc = stack.enter_context(tile.TileContext(nc))
        cpool = stack.enter_context(tc.tile_pool(name="const", bufs=1))
        v_pool = stack.enter_context(tc.tile_pool(name="vfull", bufs=1))
        ot_pool = stack.enter_context(tc.tile_pool(name="oT", bufs=1))
        qt_pool = stack.enter_context(tc.tile_pool(name="qt", bufs=2))
        kt_pool = stack.enter_context(tc.tile_pool(name="kt", bufs=2))
        wqk_pool = stack.enter_context(tc.tile_pool(name="wqk", bufs=2))
        bias_pool = stack.enter_context(tc.tile_pool(name="bias", bufs=4))
        if True:
            # ---- constants ----
            ident = cpool.tile([NP, NP], F32, name="ident")
            masks.make_identity(nc, ident[:])
            ones_row = cpool.tile([1, NP], F32, name="ones_row")
            nc.vector.memset(ones_row[:], 1.0)
            ones_col = cpool.tile([NP, 1], BF16, name="ones_col")
            nc.vector.memset(ones_col[:], 1.0)
            ones64f = cpool.tile([1, 64], F32, name="ones64f")
            nc.vector.memset(ones64f[:], 1.0)
            # ind65[h, r] = 1.0 iff the half owning output row r has its
            # reciprocal on partition h (0 or 64); rows 1..63 are zero so the
            # zero-initialized filler rows of rec65 contribute nothing.
            ind65f = cpool.tile([65, NP], F32, name="ind65f")
            nc.vector.memset(ind65f[:], 0.0)
            nc.sync.dma_start(ind65f[0:1, 0:64], ones64f[:])
            nc.sync.dma_start(ind65f[64:65, 64:NP], ones64f[:])
            ind65 = cpool.tile([65, NP], F32R, name="ind65")
            nc.vector.tensor_copy(ind65[:], ind65f[:])
            # persistent reciprocal staging rows (0 and 64); filler rows stay 0
            rec65f = cpool.tile([65, 512], F32, name="rec65f")
            nc.vector.memset(rec65f[:], 0.0)
            rec65 = cpool.tile([65, 512], F32R, name="rec65")
            nc.vector.tensor_copy(rec65[:], rec65f[:])
            recBf = cpool.tile([1, 512], F32, name="recBf")

            # b_v and b_out broadcast to [128, D] via K=1 ones matmul
            bv_row_sb = cpool.tile([1, D], F32, name="bv_row_sb")
            nc.sync.dma_start(bv_row_sb[:], bq_row[:, 2 * D:3 * D])
            bo_row_sb = cpool.tile([1, D], F32, name="bo_row_sb")
            nc.sync.dma_start(bo_row_sb[:], bout_row[:])
            bv_bc = cpool.tile([NP, D], F32, name="bv_bc")
            bo_bc = cpool.tile([NP, D], F32, name="bo_bc")
            with tc.tile_pool(name="bc_ps", bufs=2, space="PSUM") as bc_ps_pool:
                for dst, src in ((bv_bc, bv_row_sb), (bo_bc, bo_row_sb)):
                    for c in range(2):
                        ps = bc_ps_pool.tile([NP, 512], F32, name="bcps", tag="bcps")
                        nc.tensor.matmul(ps[:], ones_row[:], src[:, c * 512:(c + 1) * 512])
                        nc.vector.tensor_copy(dst[:, c * 512:(c + 1) * 512], ps[:])

            # ---- long-lived SBUF tensors ----
            vfull = v_pool.tile([NP, KT * D], BF16, name="vfull")
            v_v = vfull.rearrange("p (i f) -> p i f", i=KT)  # [tok128, ktile, feat]
            oT = [ot_pool.tile([NP, TQ], BF16, name=f"oT{p}") for p in range(PAIRS)]

            with tc.tile_pool(name="xT", bufs=1) as xT_pool, \
                 tc.tile_pool(name="proj_ps", bufs=1, space="PSUM") as proj_pool:
                xT = xT_pool.tile([NP, DC * T], F32R, name="xT")
                xT_v = xT.rearrange("p (dc t) -> p dc t", dc=DC)

                qt_tiles, kt_tiles = {}, {}

                def alloc_pair(p):
                    """DMA W/bias for pair p and allocate its q^T/k^T tiles."""
                    wt = wqk_pool.tile([NP, DC * 256], F32R, name=f"wqk{p}", tag="wqk")
                    wt_v = wt.rearrange("r (dc f) -> r dc f", dc=DC)
                    nc.sync.dma_start(wt_v[:, :, 0:128],
                                      wq_view[:, :, p * NP:(p + 1) * NP])
                    nc.sync.dma_start(wt_v[:, :, 128:256],
                                      wq_view[:, :, D + p * NP:D + (p + 1) * NP])
                    bq = bias_pool.tile([NP, 1], F32, name=f"bq{p}", tag="bq")
                    nc.sync.dma_start(bq[:], bq_col[p * NP:(p + 1) * NP, :])
                    bk = bias_pool.tile([NP, 1], F32, name=f"bk{p}", tag="bk")
                    nc.sync.dma_start(bk[:], bq_col[D + p * NP:D + (p + 1) * NP, :])
                    qt_tiles[p] = qt_pool.tile([NP, TQ], BF16, name=f"qt{p}", tag="qt")
                    kt_tiles[p] = kt_pool.tile([NP, T], BF16, name=f"kt{p}", tag="kt")
                    return wt_v, bq, bk

                def proj_chunk(p, wt_v, bq, bk, chunk):
                    """One 512-token chunk of pair-p Q^T or K^T projection."""
                    is_k = chunk >= QC
                    j = chunk - QC if is_k else chunk
                    off, dst, bias = ((128, kt_tiles[p], bk) if is_k
                                      else (0, qt_tiles[p], bq))
                    ps = proj_pool.tile([NP, 512], F32, name=f"pj{p}_{chunk}", tag="pj")
                    for dc in range(DC):
                        nc.tensor.matmul(
                            ps[:], wt_v[:, dc, off:off + 128],
                            xT_v[:, dc, j * 512:(j + 1) * 512],
                            start=(dc == 0), stop=(dc == DC - 1))
                    nc.vector.tensor_scalar_add(
                        dst[:, j * 512:(j + 1) * 512], ps[:], bias[:])

                pair_state = {}

                # ---- Phase A+B: transpose x, compute V (wv halves c-outer),
                # pair-0 Q/K projections interleaved into the second pass ----
                with ExitStack() as hstack:
                    hstack.enter_context(nc.named_scope("head"))
                    wv_pool = hstack.enter_context(tc.tile_pool(name="wv", bufs=1))
                    xl_pool = hstack.enter_context(tc.tile_pool(name="xload", bufs=2))
                    tr_pool = hstack.enter_context(
                        tc.tile_pool(name="tr_ps", bufs=2, space="PSUM"))
                    v_ps_pool = hstack.enter_context(
                        tc.tile_pool(name="v_ps", bufs=3, space="PSUM"))
                    for c in range(2):
                        wv = wv_pool.tile([NP, DC * 512], F32R, name=f"wv{c}", tag="wv")
                        wv_v = wv.rearrange("p (dc f) -> p dc f", dc=DC)
                        nc.sync.dma_start(
                            wv_v[:], wq_view[:, :, 2 * D + c * 512:2 * D + (c + 1) * 512])
                        for ti in range(KT):
                            if c == 0:
                                xt = xl_pool.tile([NP, D], F32, name=f"xt{ti}", tag="xt")
                                nc.sync.dma_start(xt[:], x_io[ti * NP:(ti + 1) * NP, :])
                                ps = tr_pool.tile([NP, D], F32, name=f"trps{ti}", tag="trps")
                                for dc in range(DC):
                                    nc.tensor.transpose(
                                        ps[:, dc * NP:(dc + 1) * NP],
                                        xt[:, dc * NP:(dc + 1) * NP], ident[:])
                                nc.vector.tensor_copy(
                                    xT_v[:, :, ti * NP:(ti + 1) * NP],
                                    ps.rearrange("p (dc t) -> p dc t", dc=DC))
                            vps = v_ps_pool.tile([NP, 512], F32, name=f"vps{c}_{ti}", tag="vps")
                            for dc in range(DC):
                                nc.tensor.matmul(
                                    vps[:], xT_v[:, dc, ti * NP:(ti + 1) * NP],
                                    wv_v[:, dc, :],
                                    start=(dc == 0), stop=(dc == DC - 1))
                            nc.vector.tensor_add(
                                v_v[:, ti, c * 512:(c + 1) * 512], vps[:],
                                bv_bc[:, c * 512:(c + 1) * 512])
                            if c == 1:
                                if ti == 0:
                                    pair_state[0] = alloc_pair(0)
                                if ti in (2, 4, 6, 8, 10, 12):
                                    proj_chunk(0, *pair_state[0], (ti - 2) // 2)

                # ---- Phase C: fused QK projection + attention ----
                with ExitStack() as astack:
                    astack.enter_context(nc.named_scope("attn"))
                    wobf_pool = astack.enter_context(tc.tile_pool(name="wout_bf", bufs=1))
                    wol_pool = astack.enter_context(tc.tile_pool(name="wol", bufs=2))
                    pab_pool = astack.enter_context(tc.tile_pool(name="pab", bufs=5))
                    acc_pool = astack.enter_context(tc.tile_pool(name="acc", bufs=2))
                    rbcsb_pool = astack.enter_context(tc.tile_pool(name="rbcsb", bufs=2))
                    s_pool = astack.enter_context(
                        tc.tile_pool(name="s_ps", bufs=2, space="PSUM"))
                    ops_pool = astack.enter_context(
                        tc.tile_pool(name="ops_ps", bufs=2, space="PSUM"))
                    dn_pool = astack.enter_context(
                        tc.tile_pool(name="dn_ps", bufs=1, space="PSUM"))
                    wout_bf = wobf_pool.tile([NP, PAIRS * D], BF16, name="wout_bf")
                    wout_bfv = wout_bf.rearrange("r (p e) -> r p e", p=PAIRS)

                    def emit_s(p, qc, i):
                        """S^T for both heads of pair p: [128 keys, 512+512 q]."""
                        s = s_pool.tile([NP, 1024], F32, name=f"s{p}_{qc}_{i}", tag="s")
                        nc.tensor.matmul(
                            s[:, 0:512], kt_tiles[p][0:HD, i * NP:(i + 1) * NP],
                            qt_tiles[p][0:HD, qc * 512:(qc + 1) * 512])
                        nc.tensor.matmul(
                            s[:, 512:1024], kt_tiles[p][HD:NP, i * NP:(i + 1) * NP],
                            qt_tiles[p][HD:NP, qc * 512:(qc + 1) * 512])
                        return s

                    def boundary_stages(p, qc, acc, ops):
                        """Denominator -> reciprocal -> broadcast -> normalized
                        drain for block (p, qc), split into stages that run
                        interleaved with the NEXT block so nothing stalls."""
                        def stage_a():
                            dnA = dn_pool.tile([NP, 512], F32,
                                               name=f"dnA{p}_{qc}", tag="dn")
                            nc.tensor.matmul(dnA[0:1, :], ones_col[:], acc[:, 0:512])
                            nc.vector.reciprocal_approx_fast(rec65f[0:1, :], dnA[0:1, :])

                        def stage_b():
                            dnB = dn_pool.tile([NP, 512], F32,
                                               name=f"dnB{p}_{qc}", tag="dn")
                            nc.tensor.matmul(dnB[0:1, :], ones_col[:], acc[:, 512:1024])
                            nc.vector.reciprocal_approx_fast(recBf[:], dnB[0:1, :])
                            nc.sync.dma_start(rec65f[64:65, :], recBf[:])
                            with nc.allow_low_precision(reason="f32r recip bcast"):
                                nc.vector.tensor_copy(rec65[:], rec65f[:])

                        def stage_c():
                            rbc = dn_pool.tile([NP, 512], F32,
                                               name=f"rbc{p}_{qc}", tag="dn")
                            nc.tensor.matmul(rbc[:], ind65[:], rec65[:])
                            rbc_sb = rbcsb_pool.tile([NP, 512], F32,
                                                     name=f"rbs{p}_{qc}", tag="rbs")
                            nc.vector.tensor_copy(rbc_sb[:], rbc[:])
                            nc.vector.tensor_mul(
                                oT[p][:, qc * 512:(qc + 1) * 512], ops[:], rbc_sb[:])

                        return {2: stage_a, 3: stage_b, 6: stage_c}

                    blocks = [(p, qc) for p in range(PAIRS) for qc in range(QC)]
                    s_cur = emit_s(0, 0, 0)
                    pending = {}
                    for bi, (p, qc) in enumerate(blocks):
                        if qc == 0 and p + 1 < PAIRS:
                            pair_state[p + 1] = alloc_pair(p + 1)
                        ops = ops_pool.tile([NP, 512], F32, name=f"ops{p}_{qc}", tag="ops")
                        acc = acc_pool.tile([NP, 1024], BF16, name=f"acc{p}_{qc}", tag="acc")
                        for i in range(KT):
                            # software pipeline: next S is in the PE queue ahead
                            # of this iter's O so ACT never waits at boundaries
                            if i < KT - 1:
                                s_nxt = emit_s(p, qc, i + 1)
                            elif bi + 1 < len(blocks):
                                s_nxt = emit_s(*blocks[bi + 1], 0)
                            else:
                                s_nxt = None
                            pab = pab_pool.tile([NP, 1024], BF16,
                                                name=f"pab{p}_{qc}_{i}", tag="pab")
                            nc.scalar.activation(pab[:], s_cur[:], AF.Exp, scale=0.125)
                            s_cur = s_nxt
                            if i == 0:
                                nc.vector.tensor_copy(acc[:], pab[:])
                            else:
                                nc.vector.tensor_add(acc[:], acc[:], pab[:])
                            # two heads share the ops PSUM bank at different
                            # partition offsets -> skip bank-granular group check
                            nc.tensor.matmul(
                                ops[0:HD, :], v_v[:, i, p * NP:p * NP + HD],
                                pab[:, 0:512],
                                start=(i == 0), stop=(i == KT - 1),
                                skip_group_check=True)
                            nc.tensor.matmul(
                                ops[HD:NP, :], v_v[:, i, p * NP + HD:(p + 1) * NP],
                                pab[:, 512:1024],
                                start=(i == 0), stop=(i == KT - 1),
                                skip_group_check=True)
                            # previous block's deferred denominator stages
                            if i in pending:
                                pending.pop(i)()
                            # keep the PE fed: next pair's projections + wout cast
                            if qc == 0 and p + 1 < PAIRS and i in (4, 5, 7, 8, 10, 12):
                                proj_chunk(p + 1, *pair_state[p + 1],
                                           (4, 5, 7, 8, 10, 12).index(i))
                            if qc == 1 and i == 9:
                                wol = wol_pool.tile([NP, D], F32, name=f"wol{p}", tag="wol")
                                nc.sync.dma_start(wol[:], wout_view[:, p, :])
                                nc.vector.tensor_copy(wout_bfv[:, p, :], wol[:])
                        if bi + 1 < len(blocks):
                            pending = boundary_stages(p, qc, acc, ops)
                        else:
                            for fn in boundary_stages(p, qc, acc, ops).values():
                                fn()

            # ---- Phase D: out projection ----
            with ExitStack() as dstack:
                dstack.enter_context(nc.named_scope("outproj"))
                f_ps_pool = dstack.enter_context(
                    tc.tile_pool(name="f_ps", bufs=4, space="PSUM"))
                f_sb_pool = dstack.enter_context(tc.tile_pool(name="f_sb", bufs=3))
                for tj in range(TQ // NP):
                    fsb = f_sb_pool.tile([NP, D], F32, name=f"fsb{tj}", tag="fsb")
                    for c in range(2):
                        ps = f_ps_pool.tile([NP, 512], F32, name=f"fps{tj}_{c}", tag="fps")
                        for p in range(PAIRS):
                            nc.tensor.matmul(
                                ps[:], oT[p][:, tj * NP:(tj + 1) * NP],
                                wout_bfv[:, p, c * 512:(c + 1) * 512],
                                start=(p == 0), stop=(p == PAIRS - 1))
                        nc.vector.tensor_add(
                            fsb[:, c * 512:(c + 1) * 512], ps[:],
                            bo_bc[:, c * 512:(c + 1) * 512])
                    nc.sync.dma_start(out_io[tj * NP:(tj + 1) * NP, :], fsb[:])

    nc.compile()
    return nc


def get_nc():
    if "nc" not in _CACHE:
        _CACHE["nc"] = _build()
    return _CACHE["nc"]


def make_in_maps(x, W_qkv, b_qkv, W_out, b_out):
    x = np.ascontiguousarray(np.asarray(x, dtype=np.float32))
    W_qkv = np.ascontiguousarray(np.asarray(W_qkv, dtype=np.float32))
    b_qkv = np.ascontiguousarray(np.asarray(b_qkv, dtype=np.float32))
    W_out = np.ascontiguousarray(np.asarray(W_out, dtype=np.float32))
    b_out = np.ascontiguousarray(np.asarray(b_out, dtype=np.float32))
    in_maps = []
    for core in range(NCORES):
        b, half = divmod(core, 2)
        xb = x[b]
        if half == 1:  # put this core's query rows first; key order is free
            xb = np.concatenate([xb[TQ:], xb[:TQ]], axis=0)
        in_maps.append({
            "x": np.ascontiguousarray(xb),
            "wqkv": W_qkv, "bqkv": b_qkv, "wout": W_out, "bout": b_out,
        })
    return in_maps


def run(in_maps, trace=False):
    from concourse.bass_utils import run_bass_kernel_spmd
    nc = get_nc()
    return run_bass_kernel_spmd(nc, in_maps, list(range(NCORES)), trace=trace)


def kernel(x, W_qkv, b_qkv, W_out, b_out):
    res = run(make_in_maps(x, W_qkv, b_qkv, W_out, b_out))
    out = np.empty((B, T, D), dtype=np.float32)
    for core in range(NCORES):
        b, half = divmod(core, 2)
        out[b, half * TQ:(half + 1) * TQ] = res.results[core]["out"]
    return out
